# revision 7
# baseline (speedup 1.0000x reference)
"""Trainium2 Bass kernel for nn_CategoricalEncoder (vq_codebook).

Computes: logits = x.reshape(B,T,N,S); idx = argmax(logits + gumbel(key42));
out[b,t,n,:] = codebook[n, idx[b,t,n], :]  (the straight-through softmax terms
cancel numerically to ~1e-7, so the exact one-hot @ codebook matmul suffices).

The Gumbel noise is a fixed constant (key 42, fixed shape/dtype); it is
precomputed on the host with the same jax backend the reference uses and
streamed to the device, where argmax/one-hot/codebook-matmul run.

Sharding: data-parallel over batch B across the 8 NeuronCores; codebook
replicated.
"""

import numpy as np

B, T, N, S, E = 32, 256, 32, 32, 128
NCORES = 8
TOK = (B // NCORES) * T  # tokens per core (1024)
PTILE = 128
NTILES = TOK // PTILE  # 8
NCHUNK = 8  # (n,s) chunks of 128 per token-tile; each covers 4 n's

_cache: dict = {}


def _gumbel() -> np.ndarray:
    """Gumbel(0,1) noise bits exactly as jax.random.categorical(key(42), ...)
    draws them on this process's default jax backend."""
    if "g" not in _cache:
        import jax
        import jax.numpy as jnp

        g = jax.random.gumbel(jax.random.key(42), (B, T, N, S), jnp.float32)
        _cache["g"] = np.asarray(g).reshape(B, T, N * S)
    return _cache["g"]


def _build_bass():
    if "nc" in _cache:
        return _cache["nc"]
    from contextlib import ExitStack

    import concourse.bacc as bacc
    import concourse.bass as bass
    import concourse.tile as tile
    from concourse import mybir
    from concourse.masks import make_identity

    fp32 = mybir.dt.float32
    nc = bacc.Bacc("TRN2", target_bir_lowering=False)
    x = nc.declare_dram_parameter("x", [TOK, N * S], fp32, isOutput=False)
    g = nc.declare_dram_parameter("g", [TOK, N * S], fp32, isOutput=False)
    cbd = nc.declare_dram_parameter("cbd", [NCHUNK, 128, 512], fp32, isOutput=False)
    out = nc.declare_dram_parameter("out", [TOK, N * E], fp32, isOutput=True)

    with ExitStack() as ctx:
        tc = ctx.enter_context(tile.TileContext(nc))
        singles = ctx.enter_context(tc.tile_pool(name="singles", bufs=1))
        ins = ctx.enter_context(tc.tile_pool(name="ins", bufs=3))
        work = ctx.enter_context(tc.tile_pool(name="work", bufs=2))
        oht_pool = ctx.enter_context(tc.tile_pool(name="oht", bufs=8))
        stage_pool = ctx.enter_context(tc.tile_pool(name="stage", bufs=2))
        pst = ctx.enter_context(tc.tile_pool(name="pst", bufs=2, space="PSUM"))
        pso = ctx.enter_context(tc.tile_pool(name="pso", bufs=4, space="PSUM"))

        identity = singles.tile([128, 128], fp32, tag="identity")
        make_identity(nc, identity)

        # Block-diagonal codebook: chunk c is [(4n x 32s)=128, (4n x 128e)=512]
        # so a single K=128 matmul contracts 4 consecutive n's at once.
        bdcb = []
        for c in range(NCHUNK):
            t_ = singles.tile([128, 512], fp32, tag=f"bdcb{c}")
            nc.sync.dma_start(out=t_, in_=cbd[c])
            bdcb.append(t_)

        for it in range(NTILES):
            rows = slice(it * PTILE, (it + 1) * PTILE)
            x_t = ins.tile([PTILE, N * S], fp32, tag="x")
            g_t = ins.tile([PTILE, N * S], fp32, tag="g")
            nc.sync.dma_start(out=x_t, in_=x[rows])
            nc.sync.dma_start(out=g_t, in_=g[rows])

            v = work.tile([PTILE, N * S], fp32, tag="v")
            nc.vector.tensor_tensor(out=v, in0=x_t, in1=g_t, op=mybir.AluOpType.add)
            v3 = v.rearrange("p (n s) -> p n s", s=S)

            m = work.tile([PTILE, N], fp32, tag="m")
            nc.vector.tensor_reduce(
                out=m, in_=v3, axis=mybir.AxisListType.X, op=mybir.AluOpType.max
            )

            onehot = work.tile([PTILE, N * S], fp32, tag="onehot")
            m_b = m.unsqueeze(2).broadcast_to([PTILE, N, S])
            nc.vector.tensor_tensor(
                out=onehot.rearrange("p (n s) -> p n s", s=S),
                in0=v3,
                in1=m_b,
                op=mybir.AluOpType.is_ge,
            )

            stage = stage_pool.tile([PTILE, N * E], fp32, tag="stage")
            for c in range(NCHUNK):
                pst_t = pst.tile([128, 128], fp32, tag="pst")
                nc.tensor.transpose(
                    pst_t, onehot[:, c * 128 : (c + 1) * 128], identity
                )
                ohT = oht_pool.tile([128, 128], fp32, tag="ohT")
                nc.scalar.copy(out=ohT, in_=pst_t)

                pso_t = pso.tile([128, 512], fp32, tag="pso")
                nc.tensor.matmul(pso_t, ohT, bdcb[c], start=True, stop=True)
                if c % 8 < 3:
                    nc.vector.tensor_copy(
                        out=stage[:, c * 512 : (c + 1) * 512], in_=pso_t
                    )
                else:
                    nc.scalar.copy(
                        out=stage[:, c * 512 : (c + 1) * 512], in_=pso_t
                    )
            nc.sync.dma_start(out=out[rows], in_=stage)

    nc.finalize()
    _cache["nc"] = nc
    return nc


def _blockdiag_cb(cb: np.ndarray) -> np.ndarray:
    """[N,S,E] -> [NCHUNK, 128, 512] block-diagonal tiles (4 n's per chunk)."""
    out = np.zeros((NCHUNK, 128, 512), dtype=np.float32)
    for c in range(NCHUNK):
        for nl in range(4):
            out[c, nl * 32 : (nl + 1) * 32, nl * 128 : (nl + 1) * 128] = cb[
                4 * c + nl
            ]
    return out


def kernel(x: np.ndarray, codebook: np.ndarray) -> np.ndarray:
    from concourse.bass_utils import run_bass_kernel_spmd

    x = np.ascontiguousarray(np.asarray(x, dtype=np.float32))
    cb = np.ascontiguousarray(np.asarray(codebook, dtype=np.float32))
    g = _gumbel()
    cbd = _blockdiag_cb(cb)

    nc = _build_bass()
    bpc = B // NCORES
    in_maps = []
    for i in range(NCORES):
        in_maps.append(
            {
                "x": x[i * bpc : (i + 1) * bpc].reshape(TOK, N * S),
                "g": g[i * bpc : (i + 1) * bpc].reshape(TOK, N * S),
                "cbd": cbd,
            }
        )
    res = run_bass_kernel_spmd(nc, in_maps, list(range(NCORES)))
    out = np.concatenate(
        [r["out"].reshape(bpc, T, N * E) for r in res.results], axis=0
    )
    return out


# revision 12
# speedup vs baseline: 1.2065x; 1.2065x over previous
"""Trainium2 Bass kernel for nn_CategoricalEncoder (vq_codebook).

Computes: logits = x.reshape(B,T,N,S); idx = argmax(logits + gumbel(key42));
out[b,t,n,:] = codebook[n, idx[b,t,n], :]  (the straight-through softmax terms
cancel numerically to ~1e-7, so the exact one-hot @ codebook matmul suffices).

The Gumbel noise is a fixed constant (key 42, fixed shape/dtype); it is
precomputed on the host with the same jax backend the reference uses and
streamed to the device, where argmax/one-hot/codebook-matmul run.

Sharding: data-parallel over batch B across the 8 NeuronCores; codebook
replicated.
"""

import numpy as np

B, T, N, S, E = 32, 256, 32, 32, 128
NCORES = 8
TOK = (B // NCORES) * T  # tokens per core (1024)
PTILE = 128
NTILES = TOK // PTILE  # 8
NCHUNK = 8  # (n,s) chunks of 128 per token-tile; each covers 4 n's

_cache: dict = {}


def _gumbel() -> np.ndarray:
    """Gumbel(0,1) noise bits exactly as jax.random.categorical(key(42), ...)
    draws them on this process's default jax backend."""
    if "g" not in _cache:
        import jax
        import jax.numpy as jnp

        g = jax.random.gumbel(jax.random.key(42), (B, T, N, S), jnp.float32)
        _cache["g"] = np.asarray(g).reshape(B, T, N * S)
    return _cache["g"]


def _build_bass():
    if "nc" in _cache:
        return _cache["nc"]
    from contextlib import ExitStack

    import concourse.bacc as bacc
    import concourse.bass as bass
    import concourse.tile as tile
    from concourse import mybir
    from concourse.masks import make_identity

    fp32 = mybir.dt.float32
    bf16 = mybir.dt.bfloat16
    nc = bacc.Bacc("TRN2", target_bir_lowering=False)
    x = nc.declare_dram_parameter("x", [TOK, N * S], fp32, isOutput=False)
    g = nc.declare_dram_parameter("g", [TOK, N * S], fp32, isOutput=False)
    cbd = nc.declare_dram_parameter("cbd", [2, NCHUNK, 128, 512], bf16, isOutput=False)
    out = nc.declare_dram_parameter("out", [TOK, N * E], fp32, isOutput=True)

    with ExitStack() as ctx:
        tc = ctx.enter_context(tile.TileContext(nc))
        singles = ctx.enter_context(tc.tile_pool(name="singles", bufs=1))
        ins = ctx.enter_context(tc.tile_pool(name="ins", bufs=3))
        work = ctx.enter_context(tc.tile_pool(name="work", bufs=2))
        oht_pool = ctx.enter_context(tc.tile_pool(name="oht", bufs=8))
        stage_pool = ctx.enter_context(tc.tile_pool(name="stage", bufs=2))
        pst = ctx.enter_context(tc.tile_pool(name="pst", bufs=2, space="PSUM"))
        pso = ctx.enter_context(tc.tile_pool(name="pso", bufs=4, space="PSUM"))

        identity = singles.tile([128, 128], fp32, tag="identity")
        make_identity(nc, identity)

        # Block-diagonal codebook: chunk c is [(4n x 32s)=128, (4n x 128e)=512]
        # so a single K=128 matmul contracts 4 consecutive n's at once.
        # bf16 hi+lo split: out = onehot @ hi + onehot @ lo (error ~2^-18).
        bdcb = [[], []]
        for h in range(2):
            for c in range(NCHUNK):
                t_ = singles.tile([128, 512], bf16, tag=f"bdcb{h}_{c}")
                nc.sync.dma_start(out=t_, in_=cbd[h, c])
                bdcb[h].append(t_)

        for it in range(NTILES):
            rows = slice(it * PTILE, (it + 1) * PTILE)
            x_t = ins.tile([PTILE, N * S], fp32, tag="x")
            g_t = ins.tile([PTILE, N * S], fp32, tag="g")
            nc.sync.dma_start(out=x_t, in_=x[rows])
            nc.sync.dma_start(out=g_t, in_=g[rows])

            v = work.tile([PTILE, N * S], fp32, tag="v")
            nc.vector.tensor_tensor(out=v, in0=x_t, in1=g_t, op=mybir.AluOpType.add)
            v3 = v.rearrange("p (n s) -> p n s", s=S)

            m = work.tile([PTILE, N], fp32, tag="m")
            nc.vector.tensor_reduce(
                out=m, in_=v3, axis=mybir.AxisListType.X, op=mybir.AluOpType.max
            )

            onehot = work.tile([PTILE, N * S], fp32, tag="onehot")
            m_b = m.unsqueeze(2).broadcast_to([PTILE, N, S])
            nc.vector.tensor_tensor(
                out=onehot.rearrange("p (n s) -> p n s", s=S),
                in0=v3,
                in1=m_b,
                op=mybir.AluOpType.is_ge,
            )

            stage = stage_pool.tile([PTILE, N * E], fp32, tag="stage")
            for c in range(NCHUNK):
                pst_t = pst.tile([128, 128], fp32, tag="pst")
                nc.tensor.transpose(
                    pst_t, onehot[:, c * 128 : (c + 1) * 128], identity
                )
                ohT = oht_pool.tile([128, 128], bf16, tag="ohT")
                nc.scalar.copy(out=ohT, in_=pst_t)

                pso_t = pso.tile([128, 512], fp32, tag="pso")
                nc.tensor.matmul(pso_t, ohT, bdcb[0][c], start=True, stop=False)
                nc.tensor.matmul(pso_t, ohT, bdcb[1][c], start=False, stop=True)
                if c % 8 < 3:
                    nc.vector.tensor_copy(
                        out=stage[:, c * 512 : (c + 1) * 512], in_=pso_t
                    )
                else:
                    nc.scalar.copy(
                        out=stage[:, c * 512 : (c + 1) * 512], in_=pso_t
                    )
            nc.sync.dma_start(out=out[rows], in_=stage)

    nc.finalize()
    _cache["nc"] = nc
    return nc


def _blockdiag_cb(cb: np.ndarray) -> np.ndarray:
    """[N,S,E] -> [2, NCHUNK, 128, 512] bf16 block-diagonal hi/lo tiles."""
    import ml_dtypes

    bf16 = ml_dtypes.bfloat16
    full = np.zeros((NCHUNK, 128, 512), dtype=np.float32)
    for c in range(NCHUNK):
        for nl in range(4):
            full[c, nl * 32 : (nl + 1) * 32, nl * 128 : (nl + 1) * 128] = cb[
                4 * c + nl
            ]
    hi = full.astype(bf16)
    lo = (full - hi.astype(np.float32)).astype(bf16)
    return np.stack([hi, lo])


def kernel(x: np.ndarray, codebook: np.ndarray) -> np.ndarray:
    from concourse.bass_utils import run_bass_kernel_spmd

    x = np.ascontiguousarray(np.asarray(x, dtype=np.float32))
    cb = np.ascontiguousarray(np.asarray(codebook, dtype=np.float32))
    g = _gumbel()
    cbd = _blockdiag_cb(cb)

    nc = _build_bass()
    bpc = B // NCORES
    in_maps = []
    for i in range(NCORES):
        in_maps.append(
            {
                "x": x[i * bpc : (i + 1) * bpc].reshape(TOK, N * S),
                "g": g[i * bpc : (i + 1) * bpc].reshape(TOK, N * S),
                "cbd": cbd,
            }
        )
    res = run_bass_kernel_spmd(nc, in_maps, list(range(NCORES)))
    out = np.concatenate(
        [r["out"].reshape(bpc, T, N * E) for r in res.results], axis=0
    )
    return out
